# revision 1
# baseline (speedup 1.0000x reference)
"""FXP BERT layer (Q16.16 int32) on 8 Trainium2 NeuronCores.

Strategy: data-parallel over batch (B=8 -> 1 batch per core). All on-device
compute is fp32 (int-valued); Q16.16 floor semantics are emulated with
ACT scale/bias + DVE magic-constant rounding. Softmax exp2 LUT is emulated
with ACT Exp on the floored LUT index (matches round(2^(16*idx/255))).

Self-contained: hardcodes B=8, S=512, H=768, heads=12, DFF=3072.
"""
import sys
import math
import numpy as np

sys.path.insert(0, "/opt/trn_rl_repo")

import concourse.bass as bass  # noqa: E402
import concourse.tile as tile  # noqa: E402
from concourse import bacc, mybir  # noqa: E402

dt = mybir.dt
AF = mybir.ActivationFunctionType
ALU = mybir.AluOpType
f32 = dt.float32
f32r = dt.float32r

B, S, H, NH, DFF = 8, 512, 768, 12, 3072
DH = H // NH            # 64
KT = H // 128           # 6 feature tiles
TT = S // 128           # 4 token tiles
FT = DFF // 128         # 24 ffn tiles

MAGIC = 12582912.0      # 1.5 * 2^23: (x + M) - M == RNE-to-int for |x| < 2^22
EPS_F = 0.4375          # floor bias, exact at ulp <= 1/16 (|x| < 2^20)
EPS_S = 0.4921875       # tighter floor bias for small-magnitude tensors
INV16 = 1.0 / 65536.0

SQ = 8192.0                                   # _c(1/sqrt(64))
CLOG2 = 94548.0                               # _c(1/log(2))
K1 = SQ / (2.0**32) * (CLOG2 / 65536.0)       # raw-score -> z units
S2 = 255.0 / (16.0 * 65536.0)                 # z -> LUT position
K1S = K1 * S2                                 # raw score -> LUT position
POSB = 255.0 - EPS_S                          # pos bias incl. floor eps
GEXP = math.log(2.0) * 16.0 / 255.0           # e = exp(GEXP * idx)
M85 = 85.0 / 65536.0                          # _c(1/768) >> 16
LN2P24 = math.log(2.0**24)

C0 = float(np.round(math.sqrt(2.0 / math.pi) * 65536.0))   # 52293
C1 = float(np.round(0.044715 * 65536.0))                   # 2930
C1PS = C1 / 65536.0 / 65536.0     # c1 * x2s * x -> c1*x^3*2^-48
C0SQ = C0 / 65536.0 / 256.0       # Square input scale: inner^2*2^-16
KR = C0 / (2.0**33)               # t' coefficient in gelu tail
N27 = 27.0 * 65536.0

_CACHE = {}


class _Emitter:
    def __init__(self, nc, tc):
        self.nc, self.tc = nc, tc
        self.scratch = tc.alloc_tile_pool(name="scratch", bufs=1)

    def sc(self, shape, tag="sc", bufs=3):
        return self.scratch.tile(list(shape), f32, name="sct", tag=tag, bufs=bufs)

    def ts_floor(self, pool, src, tag, eps=None, bufs=1, eng=None):
        """magic-round floor; if eps is None, -eps is already folded in src."""
        nc = self.nc
        eng = eng or nc.vector
        if eps is not None:
            t1 = self.sc(src.shape)
            eng.tensor_scalar(t1[:], src[:], -eps, MAGIC,
                              op0=ALU.add, op1=ALU.add)
            o = pool.tile(list(src.shape), f32, name="flo", tag=tag, bufs=bufs)
            eng.tensor_scalar(o[:], t1[:], MAGIC, None, op0=ALU.subtract)
        else:
            o = pool.tile(list(src.shape), f32, name="flo", tag=tag, bufs=bufs)
            eng.tensor_scalar(o[:], src[:], MAGIC, MAGIC,
                              op0=ALU.add, op1=ALU.subtract)
        return o

    def limbs(self, pool, src, tag, bufs=1):
        """split an int-valued fp32 tile into f32r (12-bit) high + low limbs"""
        nc = self.nc
        hi = pool.tile(list(src.shape), f32r, name="lh", tag=tag + "_h",
                       bufs=bufs)
        nc.vector.tensor_copy(hi[:], src[:])
        lo = pool.tile(list(src.shape), f32r, name="ll", tag=tag + "_l",
                       bufs=bufs)
        nc.vector.tensor_tensor(lo[:], src[:], hi[:], op=ALU.subtract)
        return hi, lo

    def evict_floor(self, pool, ps, bias_ap, tag, scale=INV16, bufs=1):
        """floor(ps*scale + bias + eps_correction) -> new tile in pool.

        bias_ap must already contain the -eps term."""
        a = self.sc((ps.shape[0], ps.shape[1]))
        self.nc.scalar.activation(a[:], ps[:], AF.Identity,
                                  bias=bias_ap, scale=scale)
        return self.ts_floor(pool, a, tag, bufs=bufs)

    def floor_small(self, pool, src, tag, scale=1.0, bufs=5):
        """all-DVE floor for tiny [1,S] chains: 2 tensor_scalar ops."""
        nc = self.nc
        t1 = self.sc((src.shape[0], src.shape[1]), tag="scs", bufs=6)
        nc.vector.tensor_scalar(t1[:], src[:], scale, -EPS_F,
                                op0=ALU.mult, op1=ALU.add)
        o = pool.tile(list(src.shape), f32, name="fls", tag=tag, bufs=bufs)
        nc.vector.tensor_scalar(o[:], t1[:], MAGIC, MAGIC,
                                op0=ALU.add, op1=ALU.subtract)
        return o


def _emit(nc):
    def din(name, shape):
        return nc.dram_tensor(name, list(shape), f32, kind="ExternalInput").ap()

    def dinr(name, shape):
        return nc.dram_tensor(name, list(shape), f32r,
                              kind="ExternalInput").ap()

    xT = din("xT", (H, S))
    wq = dinr("wqT", (H, H)); wk = dinr("wkT", (H, H))
    wv = dinr("wvT", (H, H)); wo = dinr("woT", (H, H))
    ball = din("ball", (128, 72))      # packed bias/gamma columns
    bvr = din("bvr", (1, H))
    w1 = dinr("w1T", (H, DFF))
    w2 = dinr("w2T", (DFF, H))
    ident = din("ident", (128, 128))
    h1_d = nc.dram_tensor("h1buf", [DFF, S], f32).ap()
    out_d = nc.dram_tensor("out", [H, S], f32, kind="ExternalOutput").ap()

    with tile.TileContext(nc) as tc:
        em = _Emitter(nc, tc)
        P = tc.alloc_tile_pool   # shorthand

        cpool = P(name="consts", bufs=1)

        def const_tile(val, shape, tag):
            t = cpool.tile(list(shape), f32, name="cst", tag=tag)
            nc.gpsimd.memset(t[:], val)
            return t

        ng_f = const_tile(-EPS_F, (128, 1), "ngf")
        negposb_t = const_tile(-POSB, (128, 1), "negposb")
        ones_mat = const_tile(1.0, (128, 128), "ones_mat")
        ones_row = const_tile(1.0, (1, 128), "ones_row")
        negones_row = const_tile(-1.0, (1, 128), "negones_row")
        inv16_row = const_tile(INV16, (1, 128), "inv16_row")
        b24_t = const_tile(LN2P24, (1, 1), "b24")
        ones12 = const_tile(1.0, (128, NH), "ones12")
        zeros12 = const_tile(0.0, (128, NH), "zeros12")
        consts = dict(ones_mat=ones_mat, negones_row=negones_row,
                      ones_row=ones_row, ng_f=ng_f, b24_t=b24_t,
                      inv16_row=inv16_row)

        bias_pool = P(name="biases", bufs=1)
        ball_sb = bias_pool.tile([128, 72], f32, name="ballt", tag="ball")
        nc.sync.dma_start(ball_sb[:], ball[:])
        _off = [0]

        def bias_cols(n):
            o = _off[0]
            _off[0] += n
            return [ball_sb[:, o + c:o + c + 1] for c in range(n)]

        bq_t = bias_cols(KT); bk_t = bias_cols(KT)
        bo_t = bias_cols(KT); b1_t = bias_cols(FT)
        b2_t = bias_cols(KT)
        g1_t = bias_cols(KT); l1_t = bias_cols(KT)
        g2_t = bias_cols(KT); l2_t = bias_cols(KT)
        bv_sb = bias_pool.tile([1, H], f32, name="bvt", tag="bvr")
        nc.scalar.dma_start(bv_sb[:], bvr[:])
        id_sb = bias_pool.tile([128, 128], f32, name="idt", tag="ident")
        nc.scalar.dma_start(id_sb[:], ident[:])

        # residual-stream slots rotate x -> r1 -> ln1 -> r2 -> out per tag
        res_pool = P(name="res", bufs=1)

        def res_tile(c):
            return res_pool.tile([128, S], f32, name="res", tag=f"res{c}",
                                 bufs=2)

        x_sb = []
        for c in range(KT):
            t = res_tile(c)
            nc.sync.dma_start(t[:], xT[c * 128:(c + 1) * 128, :])
            x_sb.append(t)

        vctx_pool = P(name="vctxp", bufs=1)
        qk_pool = P(name="qkp", bufs=1)

        # ---------- P1: QKV projections ----------
        xlp = P(name="xlimbs", bufs=1)
        x_hl = [em.limbs(xlp, x_sb[c], f"xh{c}") for c in range(KT)]
        wvp = P(name="wvp", bufs=1)
        wqks = P(name="wqks", bufs=1)
        pqkv = P(name="ps_qkv", bufs=1, space="PSUM")
        q_hl, k_hl, v_sb = [], [], []
        wv_sb = []
        for c in range(KT):
            t = wvp.tile([128, H], f32r, name="wvt", tag=f"wv{c}")
            nc.sync.dma_start(t[:], wv[c * 128:(c + 1) * 128, :])
            wv_sb.append(t)

        for name, wdr, b_t, dst in (("q", wq, bq_t, q_hl),
                                    ("k", wk, bk_t, k_hl)):
            for oc in range(KT):
                wc = wqks.tile([128, H], f32r, name="wqkc", tag="wqkc", bufs=3)
                nc.sync.dma_start(
                    wc[:].rearrange("p (a o) -> p a o", a=KT),
                    wdr[:, oc * 128:(oc + 1) * 128].rearrange(
                        "(a p) o -> p a o", p=128))
                ps = pqkv.tile([128, S], f32, name="qkps", tag="qkps", bufs=2)
                for kt in range(KT):
                    wsl = wc[:, kt * 128:(kt + 1) * 128]
                    nc.tensor.matmul(ps[:], wsl, x_hl[kt][0][:],
                                     start=(kt == 0), stop=False)
                    nc.tensor.matmul(ps[:], wsl, x_hl[kt][1][:],
                                     start=False, stop=(kt == KT - 1))
                fq = em.evict_floor(em.scratch, ps, b_t[oc], "qke", bufs=2)
                dst.append(em.limbs(qk_pool, fq, f"{name}{oc}"))

        # v token-major: [tok, 12*(64+1)] — a ones column per head feeds
        # the sum-of-e row of the ctx matmul
        bvf_ps = []
        for half in range(2):
            psb = pqkv.tile([128, 384], f32, name="bvf", tag=f"bvf{half}")
            nc.tensor.matmul(psb[:], ones_row[:],
                             bv_sb[:, half * 384:(half + 1) * 384],
                             start=True, stop=True)
            bvf_ps.append(psb)
        for tch in range(TT):
            vh = vctx_pool.tile([128, NH * 65], f32r, name="vth",
                                tag=f"vh{tch}")
            vl = vctx_pool.tile([128, NH * 65], f32r, name="vtl",
                                tag=f"vl{tch}")
            v_sb.append((vh, vl))
            vh_r = vh[:].rearrange("p (h c) -> p h c", c=65)
            vl_r = vl[:].rearrange("p (h c) -> p h c", c=65)
            nc.vector.tensor_copy(vh_r[:, :, 64:65], ones12[:])
            nc.vector.tensor_copy(vl_r[:, :, 64:65], zeros12[:])
            for half in range(2):
                ps = pqkv.tile([128, 384], f32, name="vps", tag="vps", bufs=2)
                for kt in range(KT):
                    xsl = slice(tch * 128, (tch + 1) * 128)
                    wsl = wv_sb[kt][:, half * 384:(half + 1) * 384]
                    nc.tensor.matmul(ps[:], x_hl[kt][0][:, xsl], wsl,
                                     start=(kt == 0), stop=False)
                    nc.tensor.matmul(ps[:], x_hl[kt][1][:, xsl], wsl,
                                     start=False, stop=(kt == KT - 1))
                a = em.sc((128, 384))
                nc.scalar.activation(a[:], ps[:], AF.Identity,
                                     bias=ng_f[:], scale=INV16)
                fl = em.ts_floor(em.scratch, a, "vfl", bufs=2)
                vf = em.sc((128, 384), tag="vfull", bufs=2)
                nc.vector.tensor_tensor(
                    vf[:], fl[:], bvf_ps[half][:], op=ALU.add)
                hsl = slice(6 * half, 6 * half + 6)
                nc.vector.tensor_copy(
                    vh_r[:, hsl, 0:64],
                    vf[:].rearrange("p (h c) -> p h c", c=64))
                nc.vector.tensor_tensor(
                    vl_r[:, hsl, 0:64],
                    vf[:].rearrange("p (h c) -> p h c", c=64),
                    vh_r[:, hsl, 0:64], op=ALU.subtract)
        pqkv.release()
        wqks.release()
        wvp.release()
        xlp.release()

        # ---------- P2: per-head row max of scores ----------
        m_pool = P(name="mrows", bufs=1)
        m_stage = m_pool.tile([1, NH * S], f32, name="mstage", tag="m_stage")
        m_all = m_pool.tile([128, NH * TT], f32, name="mall", tag="m_all")
        m_sc2 = m_pool.tile([128, NH * TT], f32, name="msc", tag="m_sc")
        mt_sb = m_pool.tile([NH * TT, 128], f32, name="mtsb", tag="mt_sb")
        psm = P(name="ps_smax", bufs=1, space="PSUM")
        for h in range(NH):
            j, base = h // 2, 64 * (h % 2)
            qh_b = q_hl[j][0][base:base + 64, :]
            ql_b = q_hl[j][1][base:base + 64, :]
            kh_b = k_hl[j][0][base:base + 64, :]
            kl_b = k_hl[j][1][base:base + 64, :]
            for qc in range(TT):
                qs = slice(qc * 128, (qc + 1) * 128)
                ps = psm.tile([128, S], f32, name="smax", tag="smax", bufs=3)
                nc.tensor.matmul(ps[:], qh_b[:, qs], kh_b[:],
                                 start=True, stop=False)
                nc.tensor.matmul(ps[:], qh_b[:, qs], kl_b[:],
                                 start=False, stop=False)
                nc.tensor.matmul(ps[:], ql_b[:, qs], kh_b[:],
                                 start=False, stop=True)
                i = h * TT + qc
                nc.vector.reduce_max(m_all[:, i:i + 1], ps[:],
                                     axis=mybir.AxisListType.X)
        nc.scalar.mul(m_sc2[:], m_all[:], K1S)
        mt_ps = psm.tile([NH * TT, 128], f32, name="mtps", tag="mt")
        nc.tensor.transpose(mt_ps[:], m_sc2[:], id_sb[:])
        nc.scalar.copy(mt_sb[:], mt_ps[:])
        nc.sync.dma_start(
            m_stage[:].rearrange("o (a i) -> o a i", a=NH * TT), mt_sb[:])
        psm.release()

        # ---------- P3: attention ----------
        psT = P(name="ps_sT", bufs=1, space="PSUM")
        pmf = P(name="ps_mf", bufs=1, space="PSUM")
        prs = P(name="ps_rs", bufs=1, space="PSUM")
        pctx = P(name="ps_ctx", bufs=1, space="PSUM")
        aws = P(name="attn_ws", bufs=1)
        ctx_sb = [None] * KT
        rs_ps = None
        ctx_ps_pair = [None, None]
        for h in range(NH):
            j, base = h // 2, 64 * (h % 2)
            qh_b = q_hl[j][0][base:base + 64, :]
            ql_b = q_hl[j][1][base:base + 64, :]
            kh_b = k_hl[j][0][base:base + 64, :]
            kl_b = k_hl[j][1][base:base + 64, :]

            mf_ps = pmf.tile([128, S], f32, name="mfps", tag="mf", bufs=1)
            nc.tensor.matmul(mf_ps[:], ones_row[:],
                             m_stage[0:1, S * h:S * (h + 1)],
                             start=True, stop=True)
            mf_sb = aws.tile([128, S], f32, name="mfsb", tag="mfsb", bufs=2)
            nc.scalar.activation(mf_sb[:], mf_ps[:], AF.Identity,
                                 bias=negposb_t[:], scale=1.0)

            ctx_ps = pctx.tile([128, S], f32, name="ctxps", tag="ctxps",
                               bufs=3)
            ctx_ps_pair[h % 2] = ctx_ps
            for c in range(TT):
                cs = slice(c * 128, (c + 1) * 128)
                ps = psT.tile([128, S], f32, name="sTps", tag="sT", bufs=3)
                nc.tensor.matmul(ps[:], kh_b[:, cs], qh_b[:],
                                 start=True, stop=False)
                nc.tensor.matmul(ps[:], kh_b[:, cs], ql_b[:],
                                 start=False, stop=False)
                nc.tensor.matmul(ps[:], kl_b[:, cs], qh_b[:],
                                 start=False, stop=True)
                z = aws.tile([128, S], f32, name="z", tag="z", bufs=2)
                nc.vector.scalar_tensor_tensor(
                    z[:], ps[:], K1S, mf_sb[:], op0=ALU.mult, op1=ALU.subtract)
                idx = em.ts_floor(aws, z, "idx", bufs=2, eng=nc.vector)
                e = aws.tile([128, S], f32r, name="e", tag="e", bufs=5)
                nc.scalar.activation(e[:], idx[:], AF.Exp, bias=0.0, scale=GEXP)
                hsl = slice(65 * h, 65 * h + 65)
                nc.tensor.matmul(ctx_ps[0:65, :], v_sb[c][0][:, hsl], e[:],
                                 start=(c == 0), stop=False)
                nc.tensor.matmul(ctx_ps[0:65, :], v_sb[c][1][:, hsl], e[:],
                                 start=False, stop=(c == TT - 1))

            # 1/sum_e broadcast into the head's half of the pair bank
            se_sb = aws.tile([1, S], f32, name="sesb", tag="sesb", bufs=1)
            nc.scalar.copy(se_sb[:], ctx_ps[64:65, :])
            se_r = aws.tile([1, S], f32, name="ser", tag="ser", bufs=1)
            nc.vector.reciprocal_approx_fast(se_r[:], se_sb[:])
            if h % 2 == 0:
                rs_ps = prs.tile([128, S], f32, name="rsps", tag="rs", bufs=1)
            nc.tensor.matmul(rs_ps[base:base + 64, :], ones_row[:, 0:64],
                             se_r[:], start=True, stop=True)

            if h % 2 == 1:
                cu = aws.tile([128, S], f32, name="cu", tag="cu", bufs=2)
                nc.scalar.copy(cu[0:64, :], ctx_ps_pair[0][0:64, :])
                nc.scalar.copy(cu[64:128, :], ctx_ps_pair[1][0:64, :])
                cn = em.sc((128, S))
                nc.vector.tensor_tensor(cn[:], cu[:], rs_ps[:], op=ALU.mult)
                ctx_sb[j] = em.ts_floor(vctx_pool, cn, f"ctx{j}", eps=EPS_F)
        for p in (aws, pctx, prs, pmf, psT):
            p.release()
        m_pool.release()
        qk_pool.release()

        # ---------- P4: WO + residual + LN1 ----------
        wo_pool = P(name="wop", bufs=1)
        wo_sb = []
        for c in range(KT):
            t = wo_pool.tile([128, H], f32r, name="wot", tag=f"wo{c}")
            nc.sync.dma_start(t[:], wo[c * 128:(c + 1) * 128, :])
            wo_sb.append(t)
        pwo = P(name="ps_wo", bufs=1, space="PSUM")
        ctxlp = P(name="ctxlimbs", bufs=1)
        ctx_hl = [em.limbs(ctxlp, ctx_sb[c], f"ch{c}") for c in range(KT)]
        r1_sb = []
        for oc in range(KT):
            ps = pwo.tile([128, S], f32, name="wops", tag="wops", bufs=2)
            for kt in range(KT):
                wsl = wo_sb[kt][:, oc * 128:(oc + 1) * 128]
                nc.tensor.matmul(ps[:], wsl, ctx_hl[kt][0][:],
                                 start=(kt == 0), stop=False)
                nc.tensor.matmul(ps[:], wsl, ctx_hl[kt][1][:],
                                 start=False, stop=(kt == KT - 1))
            w = em.evict_floor(em.scratch, ps, bo_t[oc], "woe", bufs=2)
            r = res_tile(oc)
            nc.vector.tensor_tensor(r[:], w[:], x_sb[oc][:], op=ALU.add)
            r1_sb.append(r)
        ctxlp.release()
        pwo.release()
        wo_pool.release()
        vctx_pool.release()

        pln = P(name="ps_ln1", bufs=1, space="PSUM")
        ln1_sb = _layernorm(nc, em, pln, res_tile, r1_sb, g1_t, l1_t,
                            "ln1", consts)
        pln.release()

        # ---------- P5: FFN1 + GELU ----------
        w2s = P(name="w2s", bufs=1)
        h1s = P(name="h1s", bufs=1)
        w2_t, h1_t = {}, {}

        def load_w2_pair(kt):
            t = w2s.tile([128, 2 * H], f32r, name="w2kt", tag="w2kt", bufs=3)
            nc.scalar.dma_start(
                t[:].rearrange("p (a o) -> p a o", a=2),
                w2[kt * 128:(kt + 2) * 128, :].rearrange(
                    "(a p) o -> p a o", p=128))
            w2_t[kt] = t

        def load_h1_pair(kt):
            t = h1s.tile([128, 2 * S], f32, name="h1kt", tag="h1kt", bufs=3)
            nc.scalar.dma_start(
                t[:].rearrange("p (a o) -> p a o", a=2),
                h1_d[kt * 128:(kt + 2) * 128, :].rearrange(
                    "(a p) o -> p a o", p=128))
            h1_t[kt] = t

        load_w2_pair(0)
        load_w2_pair(2)
        lnlp = P(name="ln1limbs", bufs=1)
        w1s = P(name="w1s", bufs=1)
        ph1 = P(name="ps_h1", bufs=1, space="PSUM")
        gws = P(name="gelu", bufs=1)
        ln1_hl = [em.limbs(lnlp, ln1_sb[c], f"lnh{c}") for c in range(KT)]
        wcs = {}
        for oc in range(FT):
            if oc % 4 == 0:
                wc2 = w1s.tile([128, 4 * H], f32r, name="w1c", tag="w1c",
                               bufs=2)
                deng = nc.sync if (oc // 4) % 2 == 0 else nc.scalar
                deng.dma_start(
                    wc2[:].rearrange("p (a o) -> p a o", a=4 * KT),
                    w1[:, oc * 128:(oc + 4) * 128].rearrange(
                        "(a p) o -> p a o", p=128))
                wcs = wc2
            ps = ph1.tile([128, S], f32, name="h1ps", tag="h1ps", bufs=3)
            for kt in range(KT):
                blk = 4 * kt + (oc % 4)
                wsl = wcs[:, blk * 128:(blk + 1) * 128]
                nc.tensor.matmul(ps[:], wsl, ln1_hl[kt][0][:],
                                 start=(kt == 0), stop=False)
                nc.tensor.matmul(ps[:], wsl, ln1_hl[kt][1][:],
                                 start=False, stop=(kt == KT - 1))
            xg = em.evict_floor(gws, ps, b1_t[oc], "xg", bufs=2)
            # gelu (fp32 emulation); alternate DVE/GPSIMD per chunk to
            # balance the two elementwise engines
            ve = nc.vector
            x2s = gws.tile([128, S], f32, name="x2s", tag="x2s", bufs=2)
            nc.scalar.activation(x2s[:], xg[:], AF.Square,
                                 bias=0.0, scale=1.0 / 256.0)
            u = gws.tile([128, S], f32, name="u", tag="u", bufs=2)
            ve.tensor_scalar(u[:], x2s[:], C1PS, 1.0, op0=ALU.mult,
                             op1=ALU.add)
            sarg = gws.tile([128, S], f32, name="sarg", tag="sarg", bufs=2)
            ve.tensor_tensor(sarg[:], u[:], xg[:], op=ALU.mult)
            i2 = gws.tile([128, S], f32, name="i2", tag="i2", bufs=2)
            nc.scalar.activation(i2[:], sarg[:], AF.Square, bias=0.0, scale=C0SQ)
            den = gws.tile([128, S], f32, name="den", tag="den", bufs=2)
            ve.tensor_scalar(den[:], i2[:], 9.0, N27,
                             op0=ALU.mult, op1=ALU.add)
            rden = gws.tile([128, S], f32, name="rden", tag="rden", bufs=2)
            nc.vector.reciprocal_approx_fast(rden[:], den[:])
            q1 = gws.tile([128, S], f32, name="q1", tag="q1", bufs=2)
            ve.scalar_tensor_tensor(
                q1[:], i2[:], N27, sarg[:], op0=ALU.add, op1=ALU.mult)
            tp = gws.tile([128, S], f32, name="tp", tag="tp", bufs=2)
            ve.tensor_tensor(tp[:], q1[:], rden[:], op=ALU.mult)
            rr = gws.tile([128, S], f32, name="rr", tag="rr", bufs=2)
            ve.tensor_scalar(rr[:], tp[:], KR, 0.5,
                             op0=ALU.mult, op1=ALU.add)
            hp = gws.tile([128, S], f32, name="hp", tag="hp", bufs=2)
            ve.tensor_tensor(hp[:], rr[:], xg[:], op=ALU.mult)
            h1 = em.ts_floor(gws, hp, "h1o", eps=EPS_F, bufs=3)
            nc.sync.dma_start(h1_d[oc * 128:(oc + 1) * 128, :], h1[:])
            if oc == 1:
                load_h1_pair(0)
            elif oc == 3:
                load_h1_pair(2)
        gws.release()
        ph1.release()
        w1s.release()
        lnlp.release()

        # ---------- P6: FFN2 + residual + LN2 ----------
        pf2 = P(name="ps_f2", bufs=1, space="PSUM")
        ps_f2 = [pf2.tile([128, S], f32, name="f2ps", tag=f"f2ps{oc}", bufs=1)
                 for oc in range(KT)]
        for kt in range(FT):
            if kt % 2 == 0:
                if kt not in w2_t:
                    load_w2_pair(kt)
                if kt not in h1_t:
                    load_h1_pair(kt)
                w2kt2, h1kt2 = w2_t[kt], h1_t[kt]
            sls = slice((kt % 2) * S, (kt % 2 + 1) * S)
            h1h, h1l = em.limbs(h1s, h1kt2[:, sls], "h1kl", bufs=3)
            for oc in range(KT):
                wsl = w2kt2[:, (kt % 2) * H + oc * 128:
                            (kt % 2) * H + (oc + 1) * 128]
                nc.tensor.matmul(ps_f2[oc][:], wsl, h1h[:],
                                 start=(kt == 0), stop=False)
                nc.tensor.matmul(ps_f2[oc][:], wsl, h1l[:],
                                 start=False, stop=(kt == FT - 1))
        r2_sb = []
        for oc in range(KT):
            w = em.evict_floor(em.scratch, ps_f2[oc], b2_t[oc], "f2e",
                               bufs=2)
            r = res_tile(oc)
            nc.vector.tensor_tensor(r[:], w[:], ln1_sb[oc][:], op=ALU.add)
            r2_sb.append(r)
        pf2.release()
        pln2 = P(name="ps_ln2", bufs=1, space="PSUM")
        out_sb = _layernorm(nc, em, pln2, res_tile, r2_sb, g2_t, l2_t,
                            "ln2", consts)
        for oc in range(KT):
            nc.sync.dma_start(out_d[oc * 128:(oc + 1) * 128, :], out_sb[oc][:])
        for p in (pln2, h1s, w2s, res_pool, bias_pool, cpool, em.scratch):
            p.release()

    return nc


def _layernorm(nc, em, pln, dst_tile, x_t, g_t, b_t, nm, C):
    """Q16.16 layernorm over the partition (feature) axis; x_t: 6 x [128, S]."""
    n = len(x_t)
    tmp = em.tc.alloc_tile_pool(name=nm + "_tmp", bufs=1)

    def sm(tag=None, bufs=5):
        return tmp.tile([1, S], f32, name="lns", tag=tag or (nm + "_sm"),
                        bufs=bufs)

    s_ps = pln.tile([128, S], f32, name="sps", tag=nm + "_s")
    for kt in range(n):
        nc.tensor.matmul(s_ps[:], C["ones_mat"][:], x_t[kt][:],
                         start=(kt == 0), stop=(kt == n - 1))
    mean = em.floor_small(tmp, s_ps[0:1, :], nm + "_mean", scale=M85, bufs=1)
    nm_ps = pln.tile([128, S], f32, name="nmps", tag=nm + "_nm")
    nc.tensor.matmul(nm_ps[:], C["negones_row"][:], mean[:],
                     start=True, stop=True)
    xc_t = []
    v_ps = pln.tile([128, S], f32, name="vps", tag=nm + "_v")
    for kt in range(n):
        xc = tmp.tile([128, S], f32, name="xc", tag=nm + f"_xc{kt}")
        nc.vector.tensor_tensor(xc[:], x_t[kt][:], nm_ps[:], op=ALU.add)
        xc_t.append(xc)
        x2 = tmp.tile([128, S], f32, name="x2", tag=nm + "_x2", bufs=2)
        nc.scalar.activation(x2[:], xc[:], AF.Square, bias=0.0,
                             scale=1.0 / 256.0)
        nc.tensor.matmul(v_ps[:], C["ones_mat"][:], x2[:],
                         start=(kt == 0), stop=(kt == n - 1))
    var = em.floor_small(tmp, v_ps[0:1, :], nm + "_var", scale=M85, bufs=1)
    # rsqrt seed: y0 = RNE(2^24 / sqrt(var)) via exp(-0.5*ln(var) + ln(2^24))
    l1 = sm()
    nc.scalar.activation(l1[:], var[:], AF.Ln, bias=0.0, scale=1.0)
    y0f = sm()
    nc.scalar.activation(y0f[:], l1[:], AF.Exp, bias=C["b24_t"][:], scale=-0.5)
    y = sm(tag=nm + "_y", bufs=3)
    nc.vector.tensor_scalar(y[:], y0f[:], MAGIC, MAGIC,
                            op0=ALU.add, op1=ALU.subtract)
    # two Newton iterations in exact fxp
    for it in range(2):
        yy_m = sm()
        nc.vector.tensor_tensor(yy_m[:], y[:], y[:], op=ALU.mult)
        yy = em.evict_floor(tmp, yy_m, C["ng_f"][0:1, :], nm + "_sm",
                            scale=INV16, bufs=5)
        xy_m = sm()
        nc.vector.tensor_tensor(xy_m[:], var[:], yy[:], op=ALU.mult)
        xy2 = em.evict_floor(tmp, xy_m, C["ng_f"][0:1, :], nm + "_sm",
                             scale=INV16, bufs=5)
        h = em.evict_floor(tmp, xy2, C["ng_f"][0:1, :], nm + "_sm",
                           scale=0.5, bufs=5)
        s3 = sm()
        nc.vector.tensor_scalar(s3[:], h[:], -1.0, 98304.0,
                                op0=ALU.mult, op1=ALU.add)
        ym = sm()
        nc.vector.tensor_tensor(ym[:], y[:], s3[:], op=ALU.mult)
        ynew = sm(tag=nm + "_y", bufs=3)
        a = em.sc((1, S))
        nc.scalar.activation(a[:], ym[:], AF.Identity,
                             bias=C["ng_f"][0:1, :], scale=INV16)
        nc.vector.tensor_scalar(ynew[:], a[:], MAGIC, MAGIC,
                                op0=ALU.add, op1=ALU.subtract)
        y = ynew
    inv_ps = pln.tile([128, S], f32, name="invps", tag=nm + "_inv")
    nc.tensor.matmul(inv_ps[:], C["inv16_row"][:], y[:], start=True, stop=True)
    outs = []
    for kt in range(n):
        tm = tmp.tile([128, S], f32, name="tm", tag=nm + "_tm", bufs=2)
        nc.vector.tensor_tensor(tm[:], xc_t[kt][:], inv_ps[:], op=ALU.mult)
        tq = em.ts_floor(tmp, tm, nm + "_tq", eps=EPS_F, bufs=2)
        o_p = em.sc((128, S))
        nc.scalar.activation(o_p[:], tq[:], AF.Identity,
                             bias=b_t[kt], scale=g_t[kt])
        o = dst_tile(kt)
        nc.vector.tensor_scalar(o[:], o_p[:], MAGIC, MAGIC,
                                op0=ALU.add, op1=ALU.subtract)
        outs.append(o)
    tmp.release()
    return outs


def _build():
    if "nc" in _CACHE:
        return _CACHE["nc"]
    nc = bacc.Bacc("TRN2", target_bir_lowering=False, debug=False,
                   num_devices=8)
    _emit(nc)
    nc.compile()
    _CACHE["nc"] = nc
    return nc


def _round12(a):
    a = a.astype(np.float64)
    out = np.zeros_like(a)
    nz = a != 0
    e = np.floor(np.log2(np.abs(a[nz])))
    ulp = np.power(2.0, e - 11)
    out[nz] = np.round(a[nz] / ulp) * ulp
    return out.astype(np.float32)


def _prep_maps(inputs):
    f = np.float32

    def T(a):
        return np.ascontiguousarray(np.asarray(a).T).astype(f)

    def TR(a):
        return _round12(np.ascontiguousarray(np.asarray(a).T).astype(f))

    def bias_col(b, n, eps=EPS_F):
        return (np.asarray(b).astype(np.float64) - eps).astype(f).reshape(
            n, 128, 1)

    def cols(b, eps=EPS_F):
        return (np.asarray(b).astype(np.float64) - eps).astype(f).reshape(-1, 128).T

    def gcols(g):
        return (np.asarray(g).astype(np.float64) / 65536.0).astype(f).reshape(-1, 128).T

    ball = np.concatenate([
        cols(inputs["bq"]), cols(inputs["bk"]), cols(inputs["bo"]),
        cols(inputs["b1"]), cols(inputs["b2"]),
        gcols(inputs["ln1_g"]), cols(inputs["ln1_b"]),
        gcols(inputs["ln2_g"]), cols(inputs["ln2_b"]),
    ], axis=1).astype(f)

    shared = {
        "wqT": TR(inputs["wq"]), "wkT": TR(inputs["wk"]),
        "wvT": TR(inputs["wv"]), "woT": TR(inputs["wo"]),
        "w1T": TR(inputs["w1"]), "w2T": TR(inputs["w2"]),
        "ball": ball,
        "bvr": np.asarray(inputs["bv"]).astype(f).reshape(1, H),
        "ident": np.eye(128, dtype=f),
    }
    x = np.asarray(inputs["x"])
    maps = []
    for b in range(B):
        m = dict(shared)
        m["xT"] = np.ascontiguousarray(x[b].T).astype(f)
        maps.append(m)
    return maps


def kernel(**inputs):
    from concourse.bass_utils import run_bass_kernel_spmd
    nc = _build()
    maps = _prep_maps(inputs)
    res = run_bass_kernel_spmd(nc, maps, list(range(B))).results
    out = np.stack([
        np.rint(res[b]["out"].astype(np.float64)).astype(np.int64).T
        for b in range(B)
    ])
    return np.clip(out, -2**31, 2**31 - 1).astype(np.int32)

